# revision 11
# baseline (speedup 1.0000x reference)
"""Trainium2 Bass kernel for nn_CorollaryResonanceBank.

Pure data-parallel over batch: 8 cores x 32 batch rows.

Per core:
  Phase 1 (memory-bound, overlapped): stream receive/transmit spikes
    (3 MB per batch-pair); DVE window-sums T=2048 -> 128 time bins
    (exact fp32); one fused PE matmul per tensor reduces over the 128
    (ear,channel) rows with host-folded coefficients, producing both the
    conv1x1 drive (conv weights folded) and the 3 trace channels used for
    the normalization scale, straight into PSUM. The scale is per batch,
    so the whole normalization chain (abs-max -> clamp(1) -> reciprocal ->
    partition-broadcast -> fused scale+bias into the (r, t, b) drive
    layout, with f folded in for the V-form scan) runs per pair, fully
    overlapped with streaming.
  Phase 2: 128-step resonate-and-fire scan in the V = f*v form
    (q(t) = dec*V(t-1) + f*I(t) is spike-state-free), 5 fused DVE ops
    per step ordered so the off-chain V/q ops fill the write-ack gaps
    of the dependent s2 -> spike -> reset chain; spikes land in SBUF,
    mean-reduced at the end. The drive is pre-scaled by f (folded into
    the normalization scale+bias), which is all the V-form consumes.
"""

import numpy as np

B, C, T, R, TB = 256, 64, 2048, 16, 128
W = T // TB            # 16 samples per time bin
NCORES = 8
BS = B // NCORES       # 32 batch rows per core
NPAIR = BS // 2        # 16 batch pairs per core

SCAN_SPLIT = "v3"      # "v3" (split3 V-form) | "dve" | "split"
POOL_SPLIT = False     # first halving stage of window-pool on gpsimd

_runner = None


def _softplus(x):
    return np.log1p(np.exp(np.float64(x)))


def _sigmoid(x):
    return 1.0 / (1.0 + np.exp(-np.float64(x)))


def _build_consts(conv_w, conv_b, frequency, decay, threshold,
                  transmit_weight, receive_weight):
    conv_w = np.float64(conv_w)
    conv_b = np.float64(conv_b)
    sp_r = _softplus(receive_weight)
    sp_t = _softplus(transmit_weight)
    freq = 0.02 + 0.18 * _sigmoid(frequency)
    dec = 0.8 + 0.18 * _sigmoid(decay)
    thr = 0.35 + 0.75 * _sigmoid(threshold)
    chan = np.linspace(-1.0, 1.0, C)

    # Per-row coefficients for the 3 trace channels (sd, az, el), with the
    # 1/W window-mean folded in (exact: power-of-two scaling).
    coefR = np.zeros((2 * C, 3))
    coefR[:, 0] = 0.5 * sp_r / W
    coefR[0:C, 1] = 1.0 / W
    coefR[C:, 1] = -1.0 / W
    coefR[0:C, 2] = chan / W
    coefR[C:, 2] = chan / W
    coefT = np.zeros((C, 3))
    coefT[:, 0] = -sp_t / W

    def slot(coef):
        # lhsT block (K, 35): cols 0-15 drive (conv folded), 32-34 traces
        out = np.zeros((coef.shape[0], 35))
        out[:, 0:16] = coef @ conv_w.T
        out[:, 32:35] = coef
        return out

    wr = slot(coefR)                      # (128, 35) both batch slots
    wt = np.zeros((128, 99))
    wt[0:C, 0:35] = slot(coefT)           # pair's even batch -> rows 0-98
    wt[C:, 64:99] = slot(coefT)
    scal = np.stack([dec, -freq, freq, thr, -thr,
                     1.0 - freq * freq, -(freq * freq)], axis=1)  # (16, 7)
    biasf = np.zeros((128, 1))
    biasf[0:16, 0] = conv_b
    biasf[64:80, 0] = conv_b
    f128 = np.zeros((128, 1))
    f128[0:16, 0] = freq
    f128[64:80, 0] = freq
    fbias = biasf * f128
    return (wr.astype(np.float32), wt.astype(np.float32),
            scal.astype(np.float32), biasf.astype(np.float32),
            f128.astype(np.float32), fbias.astype(np.float32))


def _build_nc(scan_split=None, pool_split=None):
    import concourse.bass as bass
    import concourse.tile as tile
    from concourse import bacc, mybir, bass_isa

    if scan_split is None:
        scan_split = SCAN_SPLIT
    if pool_split is None:
        pool_split = POOL_SPLIT
    f32 = mybir.dt.float32
    Alu = mybir.AluOpType
    X = mybir.AxisListType.X

    nc = bacc.Bacc("TRN2")
    rcv = nc.dram_tensor("receive", [BS, 2, C, T], f32, kind="ExternalInput").ap()
    tms = nc.dram_tensor("transmit", [BS, C, T], f32, kind="ExternalInput").ap()
    wr_d = nc.dram_tensor("wr", [128, 35], f32, kind="ExternalInput").ap()
    wt_d = nc.dram_tensor("wt", [128, 99], f32, kind="ExternalInput").ap()
    scal_d = nc.dram_tensor("scal", [16, 7], f32, kind="ExternalInput").ap()
    biasf_d = nc.dram_tensor("biasf", [128, 1], f32, kind="ExternalInput").ap()
    f128_d = nc.dram_tensor("f128", [128, 1], f32, kind="ExternalInput").ap()
    fbias_d = nc.dram_tensor("fbias", [128, 1], f32, kind="ExternalInput").ap()
    out_d = nc.dram_tensor("out", [16, BS], f32, kind="ExternalOutput").ap()

    rcv_v = rcv.rearrange("b i c t -> b (i c) t")              # (32, 128, 2048)
    tm_v = tms.rearrange("(p two) c t -> p (two c) t", two=2)  # (16, 128, 2048)

    with tile.TileContext(nc) as tc:
        with (
            tc.tile_pool(name="io", bufs=3) as io,
            tc.tile_pool(name="pp", bufs=3) as ppool,
            tc.tile_pool(name="small", bufs=3) as small,
            tc.tile_pool(name="persist", bufs=1) as persist,
            tc.tile_pool(name="psum", bufs=4, space="PSUM") as psum,
        ):
            wr_sb = persist.tile([128, 35], f32)
            nc.scalar.dma_start(wr_sb[:], wr_d[:])
            wt_sb = persist.tile([128, 99], f32)
            nc.scalar.dma_start(wt_sb[:], wt_d[:])
            scal_sb = persist.tile([16, 7], f32)
            nc.scalar.dma_start(scal_sb[:], scal_d[:])
            biasf_sb = persist.tile([128, 1], f32)
            nc.scalar.dma_start(biasf_sb[:], biasf_d[:])
            f128_sb = persist.tile([128, 1], f32)
            nc.scalar.dma_start(f128_sb[:], f128_d[:])
            fbias_sb = persist.tile([128, 1], f32)
            nc.scalar.dma_start(fbias_sb[:], fbias_d[:])

            drive = persist.tile([16, TB, BS], f32)
            spbuf = persist.tile([16, TB, BS], f32)
            pooled = persist.tile([99, NPAIR, TB], f32)
            m_all = persist.tile([3, BS], f32)
            mr = persist.tile([3, BS], f32)
            srow = persist.tile([1, BS], f32)
            rrow = persist.tile([1, BS], f32)
            srep = persist.tile([128, BS], f32)
            if scan_split == "v3":
                # fold f into scale+bias: scan V-form only consumes f*I
                srepf = persist.tile([128, BS], f32)
                sca_t, bia_t = srepf, fbias_sb
            else:
                sca_t, bia_t = srep, biasf_sb

            # ---- Phase 1: stream, window-pool, fused reduce, normalize ----
            for p in range(NPAIR):
                rv0 = io.tile([128, T], f32, tag="rv0")
                nc.sync.dma_start(rv0[:], rcv_v[2 * p])
                rv1 = io.tile([128, T], f32, tag="rv1")
                nc.scalar.dma_start(rv1[:], rcv_v[2 * p + 1])
                tm = io.tile([128, T], f32, tag="tm")
                nc.gpsimd.dma_start(tm[:], tm_v[p])

                def window_pool(big, tag):
                    outp = ppool.tile([128, TB], f32, tag=tag)
                    if pool_split:
                        h = ppool.tile([128, T // 2], f32, tag=tag + "_h")
                        pairs = big.rearrange("p (x two) -> p x two", two=2)
                        nc.gpsimd.tensor_add(h[:], pairs[:, :, 0],
                                             pairs[:, :, 1])
                        nc.vector.tensor_reduce(
                            out=outp[:],
                            in_=h.rearrange("p (w q) -> p w q", q=W // 2),
                            axis=X, op=Alu.add)
                    else:
                        nc.vector.tensor_reduce(
                            out=outp[:],
                            in_=big.rearrange("p (w q) -> p w q", q=W),
                            axis=X, op=Alu.add)
                    return outp

                rv0p = window_pool(rv0, "rv0p")
                rv1p = window_pool(rv1, "rv1p")
                tmp = window_pool(tm, "tmp")

                ps = psum.tile([99, TB], f32)
                nc.tensor.matmul(ps[0:35, :], wr_sb[:], rv0p[:],
                                 start=True, stop=False, skip_group_check=True)
                nc.tensor.matmul(ps[64:99, :], wr_sb[:], rv1p[:],
                                 start=True, stop=False, skip_group_check=True)
                nc.tensor.matmul(ps[0:99, :], wt_sb[:], tmp[:],
                                 start=False, stop=True, skip_group_check=True)

                # per-pair abs-max of traces; stash pooled in SBUF (ACT)
                nc.vector.tensor_reduce(
                    out=m_all[:, 2 * p:2 * p + 1], in_=ps[32:35, :], axis=X,
                    op=Alu.max, apply_absolute_value=True)
                nc.vector.tensor_reduce(
                    out=m_all[:, 2 * p + 1:2 * p + 2], in_=ps[96:99, :],
                    axis=X, op=Alu.max, apply_absolute_value=True)
                nc.scalar.copy(pooled[:, p, :], ps[:])

                # per-pair normalization: the scale is per batch, so the
                # whole chain overlaps streaming instead of trailing it.
                bs2 = slice(2 * p, 2 * p + 2)
                nc.gpsimd.partition_all_reduce(
                    mr[:, bs2], m_all[:, bs2], channels=3,
                    reduce_op=bass_isa.ReduceOp.max)
                nc.vector.tensor_scalar(out=srow[:, bs2], in0=mr[0:1, bs2],
                                        scalar1=1.0, scalar2=None,
                                        op0=Alu.max)
                nc.vector.reciprocal(out=rrow[:, bs2], in_=srow[:, bs2])
                nc.gpsimd.partition_broadcast(srep[:, bs2], rrow[:, bs2])
                if scan_split == "v3":
                    nc.vector.tensor_scalar(
                        out=srepf[:, bs2], in0=srep[:, bs2],
                        scalar1=f128_sb[:], scalar2=None, op0=Alu.mult)
                for b in (2 * p, 2 * p + 1):
                    base = 64 if b % 2 else 0
                    nc.vector.tensor_scalar(
                        out=drive[:, :, b], in0=pooled[base:base + 16, p, :],
                        scalar1=sca_t[base:base + 16, b:b + 1],
                        scalar2=bia_t[base:base + 16, :],
                        op0=Alu.mult, op1=Alu.add)

            # ---- Phase 2: resonate-and-fire scan ----
            d_s = scal_sb[:, 0:1]
            nf_s = scal_sb[:, 1:2]
            f_s = scal_sb[:, 2:3]
            th_s = scal_sb[:, 3:4]
            nth_s = scal_sb[:, 4:5]
            omf2_s = scal_sb[:, 5:6]
            nf2_s = scal_sb[:, 6:7]

            if scan_split == "v3":
                # V(t) = f*v(t) form: q(t) = dec*V(t-1) + f*I(t) is s-free,
                # so the Pool side runs a step ahead of the 3-op DVE chain
                # (s2 -> spike -> reset). drive here holds f*I.
                s_p = [persist.tile([16, BS], f32, name=f"s3_{i}")
                       for i in range(3)]
                v_p = [persist.tile([16, BS], f32, name=f"v3_{i}")
                       for i in range(2)]
                q_p = [persist.tile([16, BS], f32, name=f"q3_{i}")
                       for i in range(3)]
                s2_t = persist.tile([16, BS], f32)
                nc.vector.memset(s_p[0][:], 0.0)
                for t in range(TB):
                    sprev = s_p[t % 3]
                    snew = s_p[(t + 1) % 3]
                    qcur = drive[:, 0, :] if t == 0 else q_p[t % 3][:]
                    vnew = v_p[t % 2]
                    # All on DVE, ordered so the off-chain V/q ops fill the
                    # write-ack gaps of the dependent s2 -> sp -> s' chain.
                    nc.vector.scalar_tensor_tensor(
                        out=s2_t[:], in0=sprev[:], scalar=omf2_s, in1=qcur,
                        op0=Alu.mult, op1=Alu.add)
                    nc.vector.scalar_tensor_tensor(
                        out=vnew[:], in0=sprev[:], scalar=nf2_s, in1=qcur,
                        op0=Alu.mult, op1=Alu.add)
                    sp = spbuf[:, t, :]
                    nc.vector.tensor_scalar(
                        out=sp, in0=s2_t[:], scalar1=th_s, scalar2=None,
                        op0=Alu.is_gt)
                    if t + 1 < TB:
                        nc.vector.scalar_tensor_tensor(
                            out=q_p[(t + 1) % 3][:], in0=vnew[:],
                            scalar=d_s, in1=drive[:, t + 1, :],
                            op0=Alu.mult, op1=Alu.add)
                    nc.vector.scalar_tensor_tensor(
                        out=snew[:], in0=sp, scalar=nth_s, in1=s2_t[:],
                        op0=Alu.mult, op1=Alu.add)
            else:
                s_a = persist.tile([16, BS], f32)
                s_b = persist.tile([16, BS], f32)
                v_t = persist.tile([16, BS], f32)
                u_t = persist.tile([16, BS], f32)
                nc.vector.memset(s_a[:], 0.0)
                nc.vector.memset(v_t[:], 0.0)

                uv_eng = nc.gpsimd if scan_split == "split" else nc.vector
                for t in range(TB):
                    cur = drive[:, t, :]
                    # u = dec*v + I_t
                    uv_eng.scalar_tensor_tensor(
                        out=u_t[:], in0=v_t[:], scalar=d_s, in1=cur,
                        op0=Alu.mult, op1=Alu.add)
                    # v' = (-f)*s + u
                    uv_eng.scalar_tensor_tensor(
                        out=v_t[:], in0=s_a[:], scalar=nf_s, in1=u_t[:],
                        op0=Alu.mult, op1=Alu.add)
                    # s2 = f*v' + s
                    nc.vector.scalar_tensor_tensor(
                        out=s_b[:], in0=v_t[:], scalar=f_s, in1=s_a[:],
                        op0=Alu.mult, op1=Alu.add)
                    # sp = (s2 > thr)
                    sp = spbuf[:, t, :]
                    nc.vector.tensor_scalar(out=sp, in0=s_b[:],
                                            scalar1=th_s,
                                            scalar2=None, op0=Alu.is_gt)
                    # s = (-thr)*sp + s2
                    nc.vector.scalar_tensor_tensor(
                        out=s_a[:], in0=sp, scalar=nth_s, in1=s_b[:],
                        op0=Alu.mult, op1=Alu.add)

            osum = persist.tile([16, BS], f32)
            nc.vector.tensor_reduce(
                out=osum[:], in_=spbuf.rearrange("p t b -> p b t"),
                axis=X, op=Alu.add)
            # mean = sum / 128 is an exact power-of-two scale; the host
            # applies it bit-identically, so the output DMA ships raw sums.
            nc.sync.dma_start(out_d[:], osum[:])

    nc.compile()
    return nc


class _Runner:
    """Compiles the Bass program once and executes it via PJRT shard_map
    across the 8 NeuronCores (mirrors bass2jax.run_bass_via_pjrt, but
    keeps the jitted callable for cheap repeat calls)."""

    def __init__(self):
        import jax
        import numpy as _np
        from jax.sharding import Mesh, PartitionSpec
        from jax.experimental.shard_map import shard_map
        import concourse.mybir as mybir
        from concourse.bass2jax import (_bass_exec_p, install_neuronx_cc_hook,
                                        partition_id_tensor)

        install_neuronx_cc_hook()
        nc = _build_nc()
        self.nc = nc

        partition_name = (nc.partition_id_tensor.name
                          if nc.partition_id_tensor else None)
        in_names, out_names, out_avals, zero_outs = [], [], [], []
        for alloc in nc.m.functions[0].allocations:
            if not isinstance(alloc, mybir.MemoryLocationSet):
                continue
            name = alloc.memorylocations[0].name
            if alloc.kind == "ExternalInput":
                if name != partition_name:
                    in_names.append(name)
            elif alloc.kind == "ExternalOutput":
                out_names.append(name)
                shape = tuple(alloc.tensor_shape)
                dtype = mybir.dt.np(alloc.dtype)
                out_avals.append(jax.core.ShapedArray(shape, dtype))
                zero_outs.append(_np.zeros(shape, dtype))
        self.in_names = list(in_names)
        self.out_names = out_names
        n_params = len(in_names)
        all_in_names = in_names + out_names
        if partition_name is not None:
            all_in_names.append(partition_name)

        def _body(*args):
            operands = list(args)
            if partition_name is not None:
                operands.append(partition_id_tensor())
            outs = _bass_exec_p.bind(
                *operands,
                out_avals=tuple(out_avals),
                in_names=tuple(all_in_names),
                out_names=tuple(out_names),
                lowering_input_output_aliases=(),
                sim_require_finite=True,
                sim_require_nnan=True,
                nc=nc,
            )
            return tuple(outs)

        devices = jax.devices()[:NCORES]
        self.mesh = Mesh(np.asarray(devices), ("core",))
        in_specs = (PartitionSpec("core"),) * (n_params + len(out_names))
        out_specs = (PartitionSpec("core"),) * len(out_names)
        self.fn = jax.jit(shard_map(_body, mesh=self.mesh, in_specs=in_specs,
                                    out_specs=out_specs, check_rep=False),
                          keep_unused=True)
        self.zero_outs = zero_outs
        self.out_avals = out_avals

    def concat_inputs(self, per_core_maps):
        return [np.concatenate([m[name] for m in per_core_maps], axis=0)
                for name in self.in_names]

    def run(self, concat_in):
        concat_zeros = [np.zeros((NCORES * z.shape[0], *z.shape[1:]), z.dtype)
                        for z in self.zero_outs]
        out_arrs = self.fn(*concat_in, *concat_zeros)
        return [np.asarray(a) for a in out_arrs]


def _get_runner():
    global _runner
    if _runner is None:
        _runner = _Runner()
    return _runner


def kernel(**inputs):
    ts = np.ascontiguousarray(np.asarray(inputs["transmit_spikes"], np.float32))
    rs = np.ascontiguousarray(np.asarray(inputs["receive_spikes"], np.float32))
    wr, wt, scal, biasf, f128, fbias = _build_consts(
        np.asarray(inputs["conv_w"]), np.asarray(inputs["conv_b"]),
        np.asarray(inputs["frequency"]), np.asarray(inputs["decay"]),
        np.asarray(inputs["threshold"]),
        np.asarray(inputs["transmit_weight"]),
        np.asarray(inputs["receive_weight"]))

    runner = _get_runner()
    per_core = []
    for cidx in range(NCORES):
        bsl = slice(cidx * BS, (cidx + 1) * BS)
        per_core.append({
            "receive": rs[bsl], "transmit": ts[bsl],
            "wr": wr, "wt": wt, "scal": scal, "biasf": biasf,
            "f128": f128, "fbias": fbias,
        })
    concat_in = runner.concat_inputs(per_core)
    outs = runner.run(concat_in)
    # single output "out": concat shape (8*16, 32) -> (8, 16, 32)
    o = outs[0].reshape(NCORES, 16, BS)
    full = np.concatenate([o[c].T for c in range(NCORES)], axis=0)
    full = full.astype(np.float32) / np.float32(TB)
    return np.ascontiguousarray(full)

